# revision 7
# baseline (speedup 1.0000x reference)
"""BinaryFactoredLinear Trainium2 kernel.

y = ((x * s2) @ sign(V_latent)) @ sign(U_latent).T * s1 + bias
  x: [4, 2048, 4096] f32, V/U_latent: [4096, 512], s1/s2/bias: [4096]

Strategy (8 cores, data-parallel over the 8192 rows of x):
 - Host folds s2 into x (xs = x*s2), transposes to [D_IN, rows] and shards
   1024 rows per core, casting to bf16 (sign matrices are exactly +-1 in
   bf16, so the only rounding is on xs and on the z intermediate).
 - Device, per core:  zT[r, rows] = Vs_kc^T @ xsT_kc    (PSUM fp32 accum)
                      yT[o, rows] = UsT_rq^T @ bf16(zT)  (PSUM fp32 accum)
                      y = yT * s1 + bias  via DVE tensor_scalar with
                      per-partition scalars (s1/bias stay fp32 exact).
 - Host gathers yT shards [4096, 1024] and transposes back.

Optional USE_LO=True runs a hi/lo bf16 split of xs and of zT (double the
matmuls) for ~fp32 accuracy.
"""

import os
import numpy as np
import ml_dtypes

import concourse.bass as bass
import concourse.tile as tile
from concourse import mybir
from concourse.bass_utils import run_bass_kernel_spmd
from concourse.vector_clock import ScopedClock

BF16 = ml_dtypes.bfloat16


MAX_WAITS_PER_INST = 1


def _spill_excess_waits(nc: bass.Bass, max_waits: int = MAX_WAITS_PER_INST):
    """The walrus build in this image rejects instructions carrying more
    than a couple of sync waits ("Too many sync wait commands",
    setupSyncWait). Engines execute their instruction stream in order, so
    hoisting excess waits onto immediately-preceding same-engine NoOps is
    semantically identical."""
    spill_id = 0
    for fn in nc.m.functions:
        for bb in fn.blocks:
            insts = bb.instructions
            out = []
            changed = False
            for ins in insts:
                si = ins.sync_info
                waits = list(si.on_wait) if si is not None else []
                if len(waits) > max_waits:
                    extra = waits[max_waits:]
                    for lo in range(0, len(extra), max_waits):
                        n = mybir.InstNoOp(
                            name=f"wspill-{spill_id}", engine=ins.engine
                        )
                        spill_id += 1
                        n.sync_info = mybir.SyncInfo(
                            on_update=[], on_wait=extra[lo : lo + max_waits]
                        )
                        out.append(n)
                    si.on_wait = waits[:max_waits]
                    changed = True
                out.append(ins)
            if changed:
                bb.instructions = out

N_CORES = 8
B, S, D_IN, D_OUT, R = 4, 2048, 4096, 4096, 512
ROWS = B * S           # 8192
RPC = ROWS // N_CORES  # 1024 rows per core
KC = D_IN // 128       # 32 contraction chunks for matmul 1
RQ = R // 128          # 4  contraction chunks for matmul 2
OC = D_OUT // 128      # 32 output chunks
BLK = 512              # row-block (PSUM free dim)
NBLK = RPC // BLK      # 2

USE_LO = os.environ.get("BFL_USE_LO", "0") == "1"


def build_program(use_lo: bool = False) -> bass.Bass:
    nc = bass.Bass()
    f32 = mybir.dt.float32
    bf16 = mybir.dt.bfloat16

    # Host-prepared layouts: per-partition-contiguous.
    xt = nc.dram_tensor("xt", [128, KC * RPC], bf16, kind="ExternalInput")
    xtlo = (
        nc.dram_tensor("xtlo", [128, KC * RPC], bf16, kind="ExternalInput")
        if use_lo
        else None
    )
    vs = nc.dram_tensor("vs", [128, KC * R], bf16, kind="ExternalInput")
    ust = nc.dram_tensor("ust", [128, RQ * D_OUT], bf16, kind="ExternalInput")
    s1 = nc.dram_tensor("s1", [128, OC], f32, kind="ExternalInput")
    biast = nc.dram_tensor("biast", [128, OC], f32, kind="ExternalInput")
    yt = nc.dram_tensor("yt", [D_OUT, RPC], f32, kind="ExternalOutput")

    xt_r = xt[:].rearrange("p (kc c) -> p kc c", kc=KC)
    xtlo_r = xtlo[:].rearrange("p (kc c) -> p kc c", kc=KC) if use_lo else None
    vs_r = vs[:].rearrange("p (kc r) -> p kc r", kc=KC)
    ust_r = ust[:].rearrange("p (rq o) -> p rq o", rq=RQ)
    yt_r = yt[:].rearrange("(oc p) c -> oc p c", p=128)

    with tile.TileContext(nc) as tc:
        with (
            tc.tile_pool(name="singles", bufs=1) as singles,
            tc.tile_pool(name="xpool", bufs=1) as xpool,
            tc.tile_pool(name="ztpool", bufs=2) as ztpool,
            tc.tile_pool(name="ypool", bufs=6) as ypool,
            tc.tile_pool(name="zpsum", bufs=4, space="PSUM") as zpsum,
            tc.tile_pool(name="ypsum", bufs=3, space="PSUM") as ypsum,
        ):
            # ---- loads (chunked so PE can start early) ----
            vs_sb = singles.tile([128, KC, R], bf16, tag="vs")
            for g in range(4):
                nc.sync.dma_start(
                    out=vs_sb[:, g * 8 : (g + 1) * 8, :],
                    in_=vs_r[:, g * 8 : (g + 1) * 8, :],
                )
            xt_sb = xpool.tile([128, KC, RPC], bf16, tag="xt")
            for g in range(4):
                nc.sync.dma_start(
                    out=xt_sb[:, g * 8 : (g + 1) * 8, :],
                    in_=xt_r[:, g * 8 : (g + 1) * 8, :],
                )
            if use_lo:
                xtlo_sb = xpool.tile([128, KC, RPC], bf16, tag="xtlo")
                for g in range(4):
                    nc.sync.dma_start(
                        out=xtlo_sb[:, g * 8 : (g + 1) * 8, :],
                        in_=xtlo_r[:, g * 8 : (g + 1) * 8, :],
                    )
            ust_sb = singles.tile([128, RQ, D_OUT], bf16, tag="ust")
            for g in range(2):
                nc.sync.dma_start(
                    out=ust_sb[:, g * 2 : (g + 1) * 2, :],
                    in_=ust_r[:, g * 2 : (g + 1) * 2, :],
                )
            s1_sb = singles.tile([128, OC], f32, tag="s1")
            nc.sync.dma_start(out=s1_sb[:], in_=s1[:])
            bias_sb = singles.tile([128, OC], f32, tag="bias")
            nc.sync.dma_start(out=bias_sb[:], in_=biast[:])

            # ---- compute ----
            for b in range(NBLK):
                cs = slice(b * BLK, (b + 1) * BLK)
                # matmul 1: zT[r, rows] += Vs[k, r]^T @ xsT[k, rows]
                zt_ps = [zpsum.tile([128, BLK], f32, tag="ztps", name=f"ztps{b}_{_}") for _ in range(RQ)]
                for kc in range(KC):
                    for rq in range(RQ):
                        nc.tensor.matmul(
                            zt_ps[rq][:],
                            vs_sb[:, kc, rq * 128 : (rq + 1) * 128],
                            xt_sb[:, kc, cs],
                            start=(kc == 0),
                            stop=(kc == KC - 1 and not use_lo),
                        )
                if use_lo:
                    for kc in range(KC):
                        for rq in range(RQ):
                            nc.tensor.matmul(
                                zt_ps[rq][:],
                                vs_sb[:, kc, rq * 128 : (rq + 1) * 128],
                                xtlo_sb[:, kc, cs],
                                start=False,
                                stop=(kc == KC - 1),
                            )
                # zT fp32 -> bf16 in SBUF (moving operand of matmul 2)
                ztb = ztpool.tile([128, RQ, BLK], mybir.dt.bfloat16, tag="ztb")
                for rq in range(RQ):
                    nc.vector.tensor_copy(ztb[:, rq, :], zt_ps[rq][:])
                if use_lo:
                    zth32 = ztpool.tile([128, RQ, BLK], f32, tag="zth32")
                    ztlo = ztpool.tile([128, RQ, BLK], mybir.dt.bfloat16, tag="ztlo")
                    for rq in range(RQ):
                        nc.scalar.copy(zth32[:, rq, :], ztb[:, rq, :])
                        nc.vector.tensor_sub(zth32[:, rq, :], zt_ps[rq][:], zth32[:, rq, :])
                        nc.vector.tensor_copy(ztlo[:, rq, :], zth32[:, rq, :])
                # matmul 2 + epilogue
                for oc in range(OC):
                    y_ps = ypsum.tile([128, BLK], f32, tag="yps")
                    for rq in range(RQ):
                        nc.tensor.matmul(
                            y_ps[:],
                            ust_sb[:, rq, oc * 128 : (oc + 1) * 128],
                            ztb[:, rq, :],
                            start=(rq == 0),
                            stop=(rq == RQ - 1 and not use_lo),
                        )
                    if use_lo:
                        for rq in range(RQ):
                            nc.tensor.matmul(
                                y_ps[:],
                                ust_sb[:, rq, oc * 128 : (oc + 1) * 128],
                                ztlo[:, rq, :],
                                start=False,
                                stop=(rq == RQ - 1),
                            )
                    y_sb = ypool.tile([128, BLK], f32, tag="ysb")
                    nc.vector.tensor_scalar(
                        y_sb[:],
                        y_ps[:],
                        s1_sb[:, oc : oc + 1],
                        bias_sb[:, oc : oc + 1],
                        op0=mybir.AluOpType.mult,
                        op1=mybir.AluOpType.add,
                    )
                    nc.scalar.dma_start(out=yt_r[oc, :, cs], in_=y_sb[:])
    _spill_excess_waits(nc)
    return nc


def _to_pdim(a: np.ndarray, nchunk: int) -> np.ndarray:
    """[nchunk*128, F] row-major -> [128, nchunk*F] with per-partition
    layout [chunk][F] (partition p holds rows {chunk*128 + p})."""
    n, f = a.shape
    assert n == nchunk * 128
    return np.ascontiguousarray(
        a.reshape(nchunk, 128, f).transpose(1, 0, 2)
    ).reshape(128, nchunk * f)


_PROG_CACHE: dict[bool, bass.Bass] = {}


def kernel(x, U_latent, V_latent, s1, s2, bias, _want_trace: bool = False):
    use_lo = USE_LO
    x = np.asarray(x, np.float32)
    s1 = np.asarray(s1, np.float32)
    s2 = np.asarray(s2, np.float32)
    bias = np.asarray(bias, np.float32)

    xs = x.reshape(ROWS, D_IN) * s2[None, :]
    xsT = np.ascontiguousarray(xs.T)  # [D_IN, ROWS] f32

    vs_host = _to_pdim(np.sign(V_latent).astype(BF16), KC)
    ust_host = _to_pdim(
        np.ascontiguousarray(np.sign(U_latent).T).astype(BF16), RQ
    )
    s1_host = np.ascontiguousarray(s1.reshape(OC, 128).T)
    bias_host = np.ascontiguousarray(bias.reshape(OC, 128).T)

    in_maps = []
    for c in range(N_CORES):
        xc = xsT[:, c * RPC : (c + 1) * RPC]  # [D_IN, RPC] f32
        hi = xc.astype(BF16)
        m = {
            "xt": _to_pdim(hi, KC),
            "vs": vs_host,
            "ust": ust_host,
            "s1": s1_host,
            "biast": bias_host,
        }
        if use_lo:
            m["xtlo"] = _to_pdim((xc - hi.astype(np.float32)).astype(BF16), KC)
        in_maps.append(m)

    if use_lo not in _PROG_CACHE:
        _PROG_CACHE[use_lo] = build_program(use_lo)
    nc = _PROG_CACHE[use_lo]

    out = run_bass_kernel_spmd(
        nc, in_maps, core_ids=list(range(N_CORES)), trace=_want_trace
    )
    y = np.empty((ROWS, D_OUT), np.float32)
    for c in range(N_CORES):
        y[c * RPC : (c + 1) * RPC, :] = out.results[c]["yt"].T
    y = y.reshape(B, S, D_OUT)
    if _want_trace:
        return y, out
    return y


# revision 8
# speedup vs baseline: 1.1067x; 1.1067x over previous
"""BinaryFactoredLinear Trainium2 kernel.

y = ((x * s2) @ sign(V_latent)) @ sign(U_latent).T * s1 + bias
  x: [4, 2048, 4096] f32, V/U_latent: [4096, 512], s1/s2/bias: [4096]

Strategy (8 cores, data-parallel over the 8192 rows of x):
 - Host folds s2 into x (xs = x*s2), transposes to [D_IN, rows] and shards
   1024 rows per core, casting to bf16 (sign matrices are exactly +-1 in
   bf16, so the only rounding is on xs and on the z intermediate).
 - Device, per core:  zT[r, rows] = Vs_kc^T @ xsT_kc    (PSUM fp32 accum)
                      yT[o, rows] = UsT_rq^T @ bf16(zT)  (PSUM fp32 accum)
                      y = yT * s1 + bias  via DVE tensor_scalar with
                      per-partition scalars (s1/bias stay fp32 exact).
 - Host gathers yT shards [4096, 1024] and transposes back.

Both 512-row blocks accumulate simultaneously (8 PSUM banks) so the PE's
per-k-chunk demand stays under the DMA rate; k-chunk loads ramp up from
small sizes so the PE starts within a few us of kernel start.

Optional USE_LO=True runs a hi/lo bf16 split of xs and of zT (double the
matmuls) for ~fp32 accuracy.
"""

import os
import numpy as np
import ml_dtypes

import concourse.bass as bass
import concourse.tile as tile
from concourse import mybir
from concourse.bass_utils import run_bass_kernel_spmd

BF16 = ml_dtypes.bfloat16


MAX_WAITS_PER_INST = 1


def _spill_excess_waits(nc: bass.Bass, max_waits: int = MAX_WAITS_PER_INST):
    """The walrus build in this image rejects instructions carrying more
    than a couple of sync waits ("Too many sync wait commands",
    setupSyncWait). Engines execute their instruction stream in order, so
    hoisting excess waits onto immediately-preceding same-engine NoOps is
    semantically identical."""
    spill_id = 0
    for fn in nc.m.functions:
        for bb in fn.blocks:
            insts = bb.instructions
            out = []
            changed = False
            for ins in insts:
                si = ins.sync_info
                waits = list(si.on_wait) if si is not None else []
                if len(waits) > max_waits:
                    extra = waits[max_waits:]
                    for lo in range(0, len(extra), max_waits):
                        n = mybir.InstNoOp(
                            name=f"wspill-{spill_id}", engine=ins.engine
                        )
                        spill_id += 1
                        n.sync_info = mybir.SyncInfo(
                            on_update=[], on_wait=extra[lo : lo + max_waits]
                        )
                        out.append(n)
                    si.on_wait = waits[:max_waits]
                    changed = True
                out.append(ins)
            if changed:
                bb.instructions = out


N_CORES = 8
B, S, D_IN, D_OUT, R = 4, 2048, 4096, 4096, 512
ROWS = B * S           # 8192
RPC = ROWS // N_CORES  # 1024 rows per core
KC = D_IN // 128       # 32 contraction chunks for matmul 1
RQ = R // 128          # 4  contraction chunks for matmul 2
OC = D_OUT // 128      # 32 output chunks
BLK = 512              # row-block (PSUM free dim)
NBLK = RPC // BLK      # 2

# Cumulative k-chunk boundaries for the ramped input DMAs.
XT_CHUNKS = [0, 1, 2, 4, 8, 16, 24, 32]
VS_CHUNKS = [0, 1, 2, 4, 8, 16, 24, 32]
# k chunks after which matmul-1 switches to quarter-major order with
# inline PSUM->SBUF copies (staggered so copies hide under the PE).
KC_TAIL = 24

USE_LO = os.environ.get("BFL_USE_LO", "0") == "1"


def build_program(use_lo: bool = False) -> bass.Bass:
    nc = bass.Bass()
    f32 = mybir.dt.float32
    bf16 = mybir.dt.bfloat16

    # Host-prepared layouts: per-partition-contiguous.
    xt = nc.dram_tensor("xt", [128, KC * RPC], bf16, kind="ExternalInput")
    xtlo = (
        nc.dram_tensor("xtlo", [128, KC * RPC], bf16, kind="ExternalInput")
        if use_lo
        else None
    )
    vs = nc.dram_tensor("vs", [128, KC * R], bf16, kind="ExternalInput")
    ust = nc.dram_tensor("ust", [128, RQ * D_OUT], bf16, kind="ExternalInput")
    s1 = nc.dram_tensor("s1", [128, OC], f32, kind="ExternalInput")
    biast = nc.dram_tensor("biast", [128, OC], f32, kind="ExternalInput")
    yt = nc.dram_tensor("yt", [D_OUT, RPC], f32, kind="ExternalOutput")

    xt_r = xt[:].rearrange("p (kc c) -> p kc c", kc=KC)
    xtlo_r = xtlo[:].rearrange("p (kc c) -> p kc c", kc=KC) if use_lo else None
    vs_r = vs[:].rearrange("p (kc r) -> p kc r", kc=KC)
    ust_r = ust[:].rearrange("p (rq o) -> p rq o", rq=RQ)
    yt_r = yt[:].rearrange("(oc p) c -> oc p c", p=128)

    with tile.TileContext(nc) as tc:
        with (
            tc.tile_pool(name="singles", bufs=1) as singles,
            tc.tile_pool(name="xpool", bufs=1) as xpool,
            tc.tile_pool(name="ztpool", bufs=1) as ztpool,
            tc.tile_pool(name="ypool", bufs=4) as ypool,
            tc.tile_pool(name="pspool", bufs=8, space="PSUM") as pspool,
        ):
            # ---- loads: xt on the sync ring, weights on the scalar ring,
            # both ramped so the PE starts early ----
            xt_sb = xpool.tile([128, KC, RPC], bf16, tag="xt")
            for g in range(len(XT_CHUNKS) - 1):
                lo, hi = XT_CHUNKS[g], XT_CHUNKS[g + 1]
                nc.sync.dma_start(
                    out=xt_sb[:, lo:hi, :], in_=xt_r[:, lo:hi, :]
                )
            vs_sb = singles.tile([128, KC, R], bf16, tag="vs")
            for g in range(len(VS_CHUNKS) - 1):
                lo, hi = VS_CHUNKS[g], VS_CHUNKS[g + 1]
                nc.scalar.dma_start(
                    out=vs_sb[:, lo:hi, :], in_=vs_r[:, lo:hi, :]
                )
            if use_lo:
                xtlo_sb = xpool.tile([128, KC, RPC], bf16, tag="xtlo")
                for g in range(len(XT_CHUNKS) - 1):
                    lo, hi = XT_CHUNKS[g], XT_CHUNKS[g + 1]
                    nc.sync.dma_start(
                        out=xtlo_sb[:, lo:hi, :], in_=xtlo_r[:, lo:hi, :]
                    )
            s1_sb = singles.tile([128, OC], f32, tag="s1")
            nc.scalar.dma_start(out=s1_sb[:], in_=s1[:])
            bias_sb = singles.tile([128, OC], f32, tag="bias")
            nc.scalar.dma_start(out=bias_sb[:], in_=biast[:])
            ust_sb = singles.tile([128, RQ, D_OUT], bf16, tag="ust")
            for g in range(2):
                nc.scalar.dma_start(
                    out=ust_sb[:, g * 2 : (g + 1) * 2, :],
                    in_=ust_r[:, g * 2 : (g + 1) * 2, :],
                )

            # ---- matmul 1: zT[b][r, rows] += Vs[k, r]^T @ xsT[k, rows]
            # both row-blocks accumulate at once across 8 PSUM banks ----
            zt_ps = {
                (b, rq): pspool.tile(
                    [128, BLK], f32, tag="ps", name=f"ztps{b}_{rq}"
                )
                for b in range(NBLK)
                for rq in range(RQ)
            }
            srcs = [(xt_sb, False)] if not use_lo else [
                (xt_sb, False),
                (xtlo_sb, True),
            ]

            def mm1(kc, b, rq):
                for src_sb, is_lo in srcs:
                    nc.tensor.matmul(
                        zt_ps[b, rq][:],
                        vs_sb[:, kc, rq * 128 : (rq + 1) * 128],
                        src_sb[:, kc, b * BLK : (b + 1) * BLK],
                        start=(kc == 0 and not is_lo),
                        stop=(kc == KC - 1 and (is_lo or not use_lo)),
                    )

            for kc in range(KC_TAIL):
                for b in range(NBLK):
                    for rq in range(RQ):
                        mm1(kc, b, rq)
            # tail: quarter-major so each zT quarter finishes (and its copy
            # starts) while the PE works on the next quarter
            ztb = ztpool.tile([128, NBLK, RQ, BLK], bf16, tag="ztb")
            for b in range(NBLK):
                for rq in range(RQ):
                    for kc in range(KC_TAIL, KC):
                        mm1(kc, b, rq)
                    nc.vector.tensor_copy(ztb[:, b, rq, :], zt_ps[b, rq][:])
            if use_lo:
                zth32 = ztpool.tile([128, NBLK, RQ, BLK], f32, tag="zth32")
                ztlo = ztpool.tile([128, NBLK, RQ, BLK], bf16, tag="ztlo")
                for b in range(NBLK):
                    for rq in range(RQ):
                        nc.scalar.copy(
                            zth32[:, b, rq, :], ztb[:, b, rq, :]
                        )
                        nc.vector.tensor_sub(
                            zth32[:, b, rq, :],
                            zt_ps[b, rq][:],
                            zth32[:, b, rq, :],
                        )
                        nc.vector.tensor_copy(
                            ztlo[:, b, rq, :], zth32[:, b, rq, :]
                        )

            # ---- matmul 2 + epilogue: yT[o, rows] = UsT^T @ zT ----
            for oc in range(OC):
                y_sb = ypool.tile([128, NBLK, BLK], f32, tag="ysb")
                for b in range(NBLK):
                    y_ps = pspool.tile(
                        [128, BLK], f32, tag="ps", name=f"yps{oc}_{b}"
                    )
                    for rq in range(RQ):
                        nc.tensor.matmul(
                            y_ps[:],
                            ust_sb[:, rq, oc * 128 : (oc + 1) * 128],
                            ztb[:, b, rq, :],
                            start=(rq == 0),
                            stop=(rq == RQ - 1 and not use_lo),
                        )
                    if use_lo:
                        for rq in range(RQ):
                            nc.tensor.matmul(
                                y_ps[:],
                                ust_sb[:, rq, oc * 128 : (oc + 1) * 128],
                                ztlo[:, b, rq, :],
                                start=False,
                                stop=(rq == RQ - 1),
                            )
                    nc.vector.tensor_scalar(
                        y_sb[:, b, :],
                        y_ps[:],
                        s1_sb[:, oc : oc + 1],
                        bias_sb[:, oc : oc + 1],
                        op0=mybir.AluOpType.mult,
                        op1=mybir.AluOpType.add,
                    )
                nc.sync.dma_start(out=yt_r[oc, :, :], in_=y_sb[:, :, :])
    _spill_excess_waits(nc)
    return nc


def _to_pdim(a: np.ndarray, nchunk: int) -> np.ndarray:
    """[nchunk*128, F] row-major -> [128, nchunk*F] with per-partition
    layout [chunk][F] (partition p holds rows {chunk*128 + p})."""
    n, f = a.shape
    assert n == nchunk * 128
    return np.ascontiguousarray(
        a.reshape(nchunk, 128, f).transpose(1, 0, 2)
    ).reshape(128, nchunk * f)


_PROG_CACHE: dict[bool, bass.Bass] = {}


def kernel(x, U_latent, V_latent, s1, s2, bias, _want_trace: bool = False):
    use_lo = USE_LO
    x = np.asarray(x, np.float32)
    s1 = np.asarray(s1, np.float32)
    s2 = np.asarray(s2, np.float32)
    bias = np.asarray(bias, np.float32)

    xs = x.reshape(ROWS, D_IN) * s2[None, :]
    xsT = np.ascontiguousarray(xs.T)  # [D_IN, ROWS] f32

    vs_host = _to_pdim(np.sign(V_latent).astype(BF16), KC)
    ust_host = _to_pdim(
        np.ascontiguousarray(np.sign(U_latent).T).astype(BF16), RQ
    )
    s1_host = np.ascontiguousarray(s1.reshape(OC, 128).T)
    bias_host = np.ascontiguousarray(bias.reshape(OC, 128).T)

    in_maps = []
    for c in range(N_CORES):
        xc = xsT[:, c * RPC : (c + 1) * RPC]  # [D_IN, RPC] f32
        hi = xc.astype(BF16)
        m = {
            "xt": _to_pdim(hi, KC),
            "vs": vs_host,
            "ust": ust_host,
            "s1": s1_host,
            "biast": bias_host,
        }
        if use_lo:
            m["xtlo"] = _to_pdim((xc - hi.astype(np.float32)).astype(BF16), KC)
        in_maps.append(m)

    if use_lo not in _PROG_CACHE:
        _PROG_CACHE[use_lo] = build_program(use_lo)
    nc = _PROG_CACHE[use_lo]

    out = run_bass_kernel_spmd(
        nc, in_maps, core_ids=list(range(N_CORES)), trace=_want_trace
    )
    y = np.empty((ROWS, D_OUT), np.float32)
    for c in range(N_CORES):
        y[c * RPC : (c + 1) * RPC, :] = out.results[c]["yt"].T
    y = y.reshape(B, S, D_OUT)
    if _want_trace:
        return y, out
    return y


# revision 12
# speedup vs baseline: 1.1283x; 1.0195x over previous
"""BinaryFactoredLinear Trainium2 kernel.

y = ((x * s2) @ sign(V_latent)) @ sign(U_latent).T * s1 + bias
  x: [4, 2048, 4096] f32, V/U_latent: [4096, 512], s1/s2/bias: [4096]

Strategy (8 cores, data-parallel over the 8192 rows of x):
 - Host folds s2 into x (xs = x*s2), transposes to [D_IN, rows] and shards
   1024 rows per core, casting to bf16 (sign matrices are exactly +-1 in
   bf16, so the only rounding is on xs and on the z intermediate).
 - Device, per core:  zT[r, rows] = Vs_kc^T @ xsT_kc    (PSUM fp32 accum)
                      yT[o, rows] = UsT_rq^T @ bf16(zT)  (PSUM fp32 accum)
                      y = yT * s1 + bias  via DVE tensor_scalar with
                      per-partition scalars (s1/bias stay fp32 exact).
 - Host gathers yT shards [4096, 1024] and transposes back.

Both 512-row blocks accumulate simultaneously (8 PSUM banks) so the PE's
per-k-chunk demand stays under the DMA rate; k-chunk loads ramp up from
small sizes so the PE starts within a few us of kernel start.

Optional USE_LO=True runs a hi/lo bf16 split of xs and of zT (double the
matmuls) for ~fp32 accuracy.
"""

import os
import numpy as np
import ml_dtypes

import concourse.bass as bass
import concourse.tile as tile
from concourse import mybir
from concourse.bass_utils import run_bass_kernel_spmd

BF16 = ml_dtypes.bfloat16


MAX_WAITS_PER_INST = 1


def _spill_excess_waits(nc: bass.Bass, max_waits: int = MAX_WAITS_PER_INST):
    """The walrus build in this image rejects instructions carrying more
    than a couple of sync waits ("Too many sync wait commands",
    setupSyncWait). Engines execute their instruction stream in order, so
    hoisting excess waits onto immediately-preceding same-engine NoOps is
    semantically identical."""
    spill_id = 0
    for fn in nc.m.functions:
        for bb in fn.blocks:
            insts = bb.instructions
            out = []
            changed = False
            for ins in insts:
                si = ins.sync_info
                waits = list(si.on_wait) if si is not None else []
                if len(waits) > max_waits:
                    extra = waits[max_waits:]
                    for lo in range(0, len(extra), max_waits):
                        n = mybir.InstNoOp(
                            name=f"wspill-{spill_id}", engine=ins.engine
                        )
                        spill_id += 1
                        n.sync_info = mybir.SyncInfo(
                            on_update=[], on_wait=extra[lo : lo + max_waits]
                        )
                        out.append(n)
                    si.on_wait = waits[:max_waits]
                    changed = True
                out.append(ins)
            if changed:
                bb.instructions = out


N_CORES = 8
B, S, D_IN, D_OUT, R = 4, 2048, 4096, 4096, 512
ROWS = B * S           # 8192
RPC = ROWS // N_CORES  # 1024 rows per core
KC = D_IN // 128       # 32 contraction chunks for matmul 1
RQ = R // 128          # 4  contraction chunks for matmul 2
OC = D_OUT // 128      # 32 output chunks
BLK = 512              # row-block (PSUM free dim)
NBLK = RPC // BLK      # 2

# Cumulative k-chunk boundaries for the ramped input DMAs.
XT_CHUNKS = [0, 1, 2, 4, 8, 12, 16, 20, 24, 28, 32]
VS_CHUNKS = [0, 1, 2, 4, 8, 16, 24, 32]
# matmul-1 k chunk whose completion releases the ust load (keeps the 4MB
# ust transfer out of the bandwidth-critical early window).
UST_AFTER_KC = 12
# k chunks after which matmul-1 switches to quarter-major order with
# inline PSUM->SBUF copies (staggered so copies hide under the PE).
KC_TAIL = 24

USE_LO = os.environ.get("BFL_USE_LO", "0") == "1"


def build_program(use_lo: bool = False) -> bass.Bass:
    nc = bass.Bass()
    f32 = mybir.dt.float32
    bf16 = mybir.dt.bfloat16

    # Host-prepared layouts: per-partition-contiguous.
    xt = nc.dram_tensor("xt", [128, KC * RPC], bf16, kind="ExternalInput")
    xtlo = (
        nc.dram_tensor("xtlo", [128, KC * RPC], bf16, kind="ExternalInput")
        if use_lo
        else None
    )
    vs = nc.dram_tensor("vs", [128, KC * R], bf16, kind="ExternalInput")
    ust = nc.dram_tensor("ust", [128, RQ * D_OUT], bf16, kind="ExternalInput")
    s1 = nc.dram_tensor("s1", [128, OC], f32, kind="ExternalInput")
    biast = nc.dram_tensor("biast", [128, OC], f32, kind="ExternalInput")
    yt = nc.dram_tensor("yt", [D_OUT, RPC], f32, kind="ExternalOutput")

    xt_r = xt[:].rearrange("p (kc c) -> p kc c", kc=KC)
    xtlo_r = xtlo[:].rearrange("p (kc c) -> p kc c", kc=KC) if use_lo else None
    vs_r = vs[:].rearrange("p (kc r) -> p kc r", kc=KC)
    ust_r = ust[:].rearrange("p (rq o) -> p rq o", rq=RQ)
    yt_r = yt[:].rearrange("(oc p) c -> oc p c", p=128)

    with tile.TileContext(nc) as tc:
        with (
            tc.tile_pool(name="singles", bufs=1) as singles,
            tc.tile_pool(name="xpool", bufs=1) as xpool,
            tc.tile_pool(name="ztpool", bufs=1) as ztpool,
            tc.tile_pool(name="ypool", bufs=4) as ypool,
            tc.tile_pool(name="pspool", bufs=8, space="PSUM") as pspool,
        ):
            # ---- loads: xt on the sync ring, weights on the scalar ring,
            # both ramped so the PE starts early ----
            xt_sb = xpool.tile([128, KC, RPC], bf16, tag="xt")
            for g in range(len(XT_CHUNKS) - 1):
                lo, hi = XT_CHUNKS[g], XT_CHUNKS[g + 1]
                nc.sync.dma_start(
                    out=xt_sb[:, lo:hi, :], in_=xt_r[:, lo:hi, :]
                )
            vs_sb = singles.tile([128, KC, R], bf16, tag="vs")
            for g in range(len(VS_CHUNKS) - 1):
                lo, hi = VS_CHUNKS[g], VS_CHUNKS[g + 1]
                nc.scalar.dma_start(
                    out=vs_sb[:, lo:hi, :], in_=vs_r[:, lo:hi, :]
                )
            if use_lo:
                xtlo_sb = xpool.tile([128, KC, RPC], bf16, tag="xtlo")
                for g in range(len(XT_CHUNKS) - 1):
                    lo, hi = XT_CHUNKS[g], XT_CHUNKS[g + 1]
                    nc.sync.dma_start(
                        out=xtlo_sb[:, lo:hi, :], in_=xtlo_r[:, lo:hi, :]
                    )
            s1_sb = singles.tile([128, OC], f32, tag="s1")
            nc.scalar.dma_start(out=s1_sb[:], in_=s1[:])
            bias_sb = singles.tile([128, OC], f32, tag="bias")
            nc.scalar.dma_start(out=bias_sb[:], in_=biast[:])
            ust_sb = singles.tile([128, RQ, D_OUT], bf16, tag="ust")
            ust_dmas = [
                nc.scalar.dma_start(
                    out=ust_sb[:, g * 2 : (g + 1) * 2, :],
                    in_=ust_r[:, g * 2 : (g + 1) * 2, :],
                )
                for g in range(2)
            ]

            # ---- matmul 1: zT[b][r, rows] += Vs[k, r]^T @ xsT[k, rows]
            # both row-blocks accumulate at once across 8 PSUM banks ----
            zt_ps = {
                (b, rq): pspool.tile(
                    [128, BLK], f32, tag="ps", name=f"ztps{b}_{rq}"
                )
                for b in range(NBLK)
                for rq in range(RQ)
            }
            srcs = [(xt_sb, False)] if not use_lo else [
                (xt_sb, False),
                (xtlo_sb, True),
            ]

            def mm1(kc, b, rq):
                last = None
                for src_sb, is_lo in srcs:
                    last = nc.tensor.matmul(
                        zt_ps[b, rq][:],
                        vs_sb[:, kc, rq * 128 : (rq + 1) * 128],
                        src_sb[:, kc, b * BLK : (b + 1) * BLK],
                        start=(kc == 0 and not is_lo),
                        stop=(kc == KC - 1 and (is_lo or not use_lo)),
                    )
                return last

            for kc in range(KC_TAIL):
                for b in range(NBLK):
                    for rq in range(RQ):
                        mm = mm1(kc, b, rq)
                if kc == UST_AFTER_KC:
                    # hold the ust stream out of the early DMA window
                    for dma in ust_dmas:
                        tile.add_dep_helper(
                            dma.ins, mm.ins, sync=True,
                            reason="delay ust load past the hot start",
                        )
            # tail: quarter-major so each zT quarter finishes (and its copy
            # starts) while the PE works on the next quarter
            ztb = ztpool.tile([128, NBLK, RQ, BLK], bf16, tag="ztb")
            for b in range(NBLK):
                for rq in range(RQ):
                    for kc in range(KC_TAIL, KC):
                        mm1(kc, b, rq)
                    nc.vector.tensor_copy(ztb[:, b, rq, :], zt_ps[b, rq][:])
            if use_lo:
                zth32 = ztpool.tile([128, NBLK, RQ, BLK], f32, tag="zth32")
                ztlo = ztpool.tile([128, NBLK, RQ, BLK], bf16, tag="ztlo")
                for b in range(NBLK):
                    for rq in range(RQ):
                        nc.scalar.copy(
                            zth32[:, b, rq, :], ztb[:, b, rq, :]
                        )
                        nc.vector.tensor_sub(
                            zth32[:, b, rq, :],
                            zt_ps[b, rq][:],
                            zth32[:, b, rq, :],
                        )
                        nc.vector.tensor_copy(
                            ztlo[:, b, rq, :], zth32[:, b, rq, :]
                        )

            # ---- matmul 2 + epilogue: yT[o, rows] = UsT^T @ zT ----
            for oc in range(OC):
                y_sb = ypool.tile([128, NBLK, BLK], f32, tag="ysb")
                for b in range(NBLK):
                    y_ps = pspool.tile(
                        [128, BLK], f32, tag="ps", name=f"yps{oc}_{b}"
                    )
                    for rq in range(RQ):
                        nc.tensor.matmul(
                            y_ps[:],
                            ust_sb[:, rq, oc * 128 : (oc + 1) * 128],
                            ztb[:, b, rq, :],
                            start=(rq == 0),
                            stop=(rq == RQ - 1 and not use_lo),
                        )
                    if use_lo:
                        for rq in range(RQ):
                            nc.tensor.matmul(
                                y_ps[:],
                                ust_sb[:, rq, oc * 128 : (oc + 1) * 128],
                                ztlo[:, b, rq, :],
                                start=False,
                                stop=(rq == RQ - 1),
                            )
                    nc.vector.tensor_scalar(
                        y_sb[:, b, :],
                        y_ps[:],
                        s1_sb[:, oc : oc + 1],
                        bias_sb[:, oc : oc + 1],
                        op0=mybir.AluOpType.mult,
                        op1=mybir.AluOpType.add,
                    )
                    if oc == OC - 1:
                        # split the final store so its first half overlaps
                        # the last matmuls (shorter kernel tail)
                        nc.sync.dma_start(
                            out=yt_r[oc, :, b * BLK : (b + 1) * BLK],
                            in_=y_sb[:, b, :],
                        )
                if oc < OC - 1:
                    nc.sync.dma_start(out=yt_r[oc, :, :], in_=y_sb[:, :, :])
    _spill_excess_waits(nc)
    return nc


def _to_pdim(a: np.ndarray, nchunk: int) -> np.ndarray:
    """[nchunk*128, F] row-major -> [128, nchunk*F] with per-partition
    layout [chunk][F] (partition p holds rows {chunk*128 + p})."""
    n, f = a.shape
    assert n == nchunk * 128
    return np.ascontiguousarray(
        a.reshape(nchunk, 128, f).transpose(1, 0, 2)
    ).reshape(128, nchunk * f)


_PROG_CACHE: dict[bool, bass.Bass] = {}


def kernel(x, U_latent, V_latent, s1, s2, bias, _want_trace: bool = False):
    use_lo = USE_LO
    x = np.asarray(x, np.float32)
    s1 = np.asarray(s1, np.float32)
    s2 = np.asarray(s2, np.float32)
    bias = np.asarray(bias, np.float32)

    xs = x.reshape(ROWS, D_IN) * s2[None, :]
    xsT = np.ascontiguousarray(xs.T)  # [D_IN, ROWS] f32

    vs_host = _to_pdim(np.sign(V_latent).astype(BF16), KC)
    ust_host = _to_pdim(
        np.ascontiguousarray(np.sign(U_latent).T).astype(BF16), RQ
    )
    s1_host = np.ascontiguousarray(s1.reshape(OC, 128).T)
    bias_host = np.ascontiguousarray(bias.reshape(OC, 128).T)

    in_maps = []
    for c in range(N_CORES):
        xc = xsT[:, c * RPC : (c + 1) * RPC]  # [D_IN, RPC] f32
        hi = xc.astype(BF16)
        m = {
            "xt": _to_pdim(hi, KC),
            "vs": vs_host,
            "ust": ust_host,
            "s1": s1_host,
            "biast": bias_host,
        }
        if use_lo:
            m["xtlo"] = _to_pdim((xc - hi.astype(np.float32)).astype(BF16), KC)
        in_maps.append(m)

    if use_lo not in _PROG_CACHE:
        _PROG_CACHE[use_lo] = build_program(use_lo)
    nc = _PROG_CACHE[use_lo]

    out = run_bass_kernel_spmd(
        nc, in_maps, core_ids=list(range(N_CORES)), trace=_want_trace
    )
    y = np.empty((ROWS, D_OUT), np.float32)
    for c in range(N_CORES):
        y[c * RPC : (c + 1) * RPC, :] = out.results[c]["yt"].T
    y = y.reshape(B, S, D_OUT)
    if _want_trace:
        return y, out
    return y


# revision 14
# speedup vs baseline: 1.1560x; 1.0246x over previous
"""BinaryFactoredLinear Trainium2 kernel.

y = ((x * s2) @ sign(V_latent)) @ sign(U_latent).T * s1 + bias
  x: [4, 2048, 4096] f32, V/U_latent: [4096, 512], s1/s2/bias: [4096]

Strategy (8 cores, data-parallel over the 8192 rows of x):
 - Host folds s2 into x (xs = x*s2), transposes to [D_IN, rows] and shards
   1024 rows per core, casting to bf16 (sign matrices are exactly +-1 in
   bf16, so the only rounding is on xs and on the z intermediate).
 - Device, per core:  zT[r, rows] = Vs_kc^T @ xsT_kc    (PSUM fp32 accum)
                      yT[o, rows] = UsT_rq^T @ bf16(zT)  (PSUM fp32 accum)
                      y = yT * s1 + bias  via DVE tensor_scalar with
                      per-partition scalars (s1/bias stay fp32 exact).
 - Host gathers yT shards [4096, 1024] and transposes back.

Both 512-row blocks accumulate simultaneously (8 PSUM banks) so the PE's
per-k-chunk demand stays under the DMA rate; k-chunk loads ramp up from
small sizes so the PE starts within a few us of kernel start.

Optional USE_LO=True runs a hi/lo bf16 split of xs and of zT (double the
matmuls) for ~fp32 accuracy.
"""

import os
import numpy as np
import ml_dtypes

import concourse.bass as bass
import concourse.tile as tile
from concourse import mybir
from concourse.bass_utils import run_bass_kernel_spmd

BF16 = ml_dtypes.bfloat16


MAX_WAITS_PER_INST = 1


def _spill_excess_waits(nc: bass.Bass, max_waits: int = MAX_WAITS_PER_INST):
    """The walrus build in this image rejects instructions carrying more
    than a couple of sync waits ("Too many sync wait commands",
    setupSyncWait). Engines execute their instruction stream in order, so
    hoisting excess waits onto immediately-preceding same-engine NoOps is
    semantically identical."""
    spill_id = 0
    for fn in nc.m.functions:
        for bb in fn.blocks:
            insts = bb.instructions
            out = []
            changed = False
            for ins in insts:
                si = ins.sync_info
                waits = list(si.on_wait) if si is not None else []
                if len(waits) > max_waits:
                    extra = waits[max_waits:]
                    for lo in range(0, len(extra), max_waits):
                        n = mybir.InstNoOp(
                            name=f"wspill-{spill_id}", engine=ins.engine
                        )
                        spill_id += 1
                        n.sync_info = mybir.SyncInfo(
                            on_update=[], on_wait=extra[lo : lo + max_waits]
                        )
                        out.append(n)
                    si.on_wait = waits[:max_waits]
                    changed = True
                out.append(ins)
            if changed:
                bb.instructions = out


N_CORES = 8
B, S, D_IN, D_OUT, R = 4, 2048, 4096, 4096, 512
ROWS = B * S           # 8192
RPC = ROWS // N_CORES  # 1024 rows per core
KC = D_IN // 128       # 32 contraction chunks for matmul 1
RQ = R // 128          # 4  contraction chunks for matmul 2
OC = D_OUT // 128      # 32 output chunks
BLK = 512              # row-block (PSUM free dim)
NBLK = RPC // BLK      # 2

# Cumulative k-chunk boundaries for the ramped input DMAs.
XT_CHUNKS = [0, 1, 2, 4, 8, 12, 16, 20, 24, 28, 32]
VS_CHUNKS = [0, 1, 2, 4, 8, 16, 24, 32]
# matmul-1 k chunk whose completion releases the ust load (keeps the 4MB
# ust transfer out of the bandwidth-critical early window).
UST_AFTER_KC = 12
# k chunks after which matmul-1 switches to quarter-major order with
# inline PSUM->SBUF copies (staggered so copies hide under the PE).
KC_TAIL = 24

USE_LO = os.environ.get("BFL_USE_LO", "0") == "1"


def build_program(use_lo: bool = False) -> bass.Bass:
    nc = bass.Bass()
    f32 = mybir.dt.float32
    bf16 = mybir.dt.bfloat16

    # Host-prepared layouts: per-partition-contiguous.
    xt = nc.dram_tensor("xt", [128, KC * RPC], bf16, kind="ExternalInput")
    xtlo = (
        nc.dram_tensor("xtlo", [128, KC * RPC], bf16, kind="ExternalInput")
        if use_lo
        else None
    )
    vs = nc.dram_tensor("vs", [128, KC * R], bf16, kind="ExternalInput")
    ust = nc.dram_tensor("ust", [128, RQ * D_OUT], bf16, kind="ExternalInput")
    s1 = nc.dram_tensor("s1", [128, OC], f32, kind="ExternalInput")
    biast = nc.dram_tensor("biast", [128, OC], f32, kind="ExternalInput")
    yt = nc.dram_tensor("yt", [D_OUT, RPC], f32, kind="ExternalOutput")

    xt_r = xt[:].rearrange("p (kc c) -> p kc c", kc=KC)
    xtlo_r = xtlo[:].rearrange("p (kc c) -> p kc c", kc=KC) if use_lo else None
    vs_r = vs[:].rearrange("p (kc r) -> p kc r", kc=KC)
    ust_r = ust[:].rearrange("p (rq o) -> p rq o", rq=RQ)
    yt_r = yt[:].rearrange("(oc p) c -> oc p c", p=128)

    with tile.TileContext(nc) as tc:
        with (
            tc.tile_pool(name="singles", bufs=1) as singles,
            tc.tile_pool(name="xpool", bufs=1) as xpool,
            tc.tile_pool(name="ztpool", bufs=1) as ztpool,
            tc.tile_pool(name="ypool", bufs=4) as ypool,
            tc.tile_pool(name="pspool", bufs=8, space="PSUM") as pspool,
        ):
            # ---- loads: xt on the sync ring, weights on the scalar ring,
            # both ramped so the PE starts early ----
            xt_sb = xpool.tile([128, KC, RPC], bf16, tag="xt")
            for g in range(len(XT_CHUNKS) - 1):
                lo, hi = XT_CHUNKS[g], XT_CHUNKS[g + 1]
                nc.sync.dma_start(
                    out=xt_sb[:, lo:hi, :], in_=xt_r[:, lo:hi, :]
                )
            vs_sb = singles.tile([128, KC, R], bf16, tag="vs")
            for g in range(len(VS_CHUNKS) - 1):
                lo, hi = VS_CHUNKS[g], VS_CHUNKS[g + 1]
                nc.scalar.dma_start(
                    out=vs_sb[:, lo:hi, :], in_=vs_r[:, lo:hi, :]
                )
            if use_lo:
                xtlo_sb = xpool.tile([128, KC, RPC], bf16, tag="xtlo")
                for g in range(len(XT_CHUNKS) - 1):
                    lo, hi = XT_CHUNKS[g], XT_CHUNKS[g + 1]
                    nc.sync.dma_start(
                        out=xtlo_sb[:, lo:hi, :], in_=xtlo_r[:, lo:hi, :]
                    )
            s1_sb = singles.tile([128, OC], f32, tag="s1")
            nc.scalar.dma_start(out=s1_sb[:], in_=s1[:])
            bias_sb = singles.tile([128, OC], f32, tag="bias")
            nc.scalar.dma_start(out=bias_sb[:], in_=biast[:])
            ust_sb = singles.tile([128, RQ, D_OUT], bf16, tag="ust")
            ust_dmas = [
                nc.scalar.dma_start(
                    out=ust_sb[:, g * 2 : (g + 1) * 2, :],
                    in_=ust_r[:, g * 2 : (g + 1) * 2, :],
                )
                for g in range(2)
            ]

            # ---- PE warm-up: dummy matmuls fill the otherwise-idle preamble
            # window so the HAM clock-gate reaches 2.4GHz before real work ----
            warm_sb = singles.tile([128, 256], bf16, tag="warm")
            nc.gpsimd.memset(warm_sb[:], 0)
            warm_ps = pspool.tile([128, 256], f32, tag="ps", name="warmps")
            for _ in range(8):
                nc.tensor.matmul(
                    warm_ps[:], warm_sb[:, 0:128], warm_sb[:, :], start=True,
                    stop=True,
                )

            # ---- matmul 1: zT[b][r, rows] += Vs[k, r]^T @ xsT[k, rows]
            # both row-blocks accumulate at once across 8 PSUM banks ----
            zt_ps = {
                (b, rq): pspool.tile(
                    [128, BLK], f32, tag="ps", name=f"ztps{b}_{rq}"
                )
                for b in range(NBLK)
                for rq in range(RQ)
            }
            srcs = [(xt_sb, False)] if not use_lo else [
                (xt_sb, False),
                (xtlo_sb, True),
            ]

            def mm1(kc, b, rq):
                last = None
                for src_sb, is_lo in srcs:
                    last = nc.tensor.matmul(
                        zt_ps[b, rq][:],
                        vs_sb[:, kc, rq * 128 : (rq + 1) * 128],
                        src_sb[:, kc, b * BLK : (b + 1) * BLK],
                        start=(kc == 0 and not is_lo),
                        stop=(kc == KC - 1 and (is_lo or not use_lo)),
                    )
                return last

            for kc in range(KC_TAIL):
                for b in range(NBLK):
                    for rq in range(RQ):
                        mm = mm1(kc, b, rq)
                if kc == UST_AFTER_KC:
                    # hold the ust stream out of the early DMA window
                    for dma in ust_dmas:
                        tile.add_dep_helper(
                            dma.ins, mm.ins, sync=True,
                            reason="delay ust load past the hot start",
                        )
            # tail: quarter-major so each zT quarter finishes (and its copy
            # starts) while the PE works on the next quarter
            ztb = ztpool.tile([128, NBLK, RQ, BLK], bf16, tag="ztb")
            for b in range(NBLK):
                for rq in range(RQ):
                    for kc in range(KC_TAIL, KC):
                        mm1(kc, b, rq)
                    nc.vector.tensor_copy(ztb[:, b, rq, :], zt_ps[b, rq][:])
            if use_lo:
                zth32 = ztpool.tile([128, NBLK, RQ, BLK], f32, tag="zth32")
                ztlo = ztpool.tile([128, NBLK, RQ, BLK], bf16, tag="ztlo")
                for b in range(NBLK):
                    for rq in range(RQ):
                        nc.scalar.copy(
                            zth32[:, b, rq, :], ztb[:, b, rq, :]
                        )
                        nc.vector.tensor_sub(
                            zth32[:, b, rq, :],
                            zt_ps[b, rq][:],
                            zth32[:, b, rq, :],
                        )
                        nc.vector.tensor_copy(
                            ztlo[:, b, rq, :], zth32[:, b, rq, :]
                        )

            # ---- matmul 2 + epilogue: yT[o, rows] = UsT^T @ zT ----
            for oc in range(OC):
                y_sb = ypool.tile([128, NBLK, BLK], f32, tag="ysb")
                for b in range(NBLK):
                    y_ps = pspool.tile(
                        [128, BLK], f32, tag="ps", name=f"yps{oc}_{b}"
                    )
                    for rq in range(RQ):
                        nc.tensor.matmul(
                            y_ps[:],
                            ust_sb[:, rq, oc * 128 : (oc + 1) * 128],
                            ztb[:, b, rq, :],
                            start=(rq == 0),
                            stop=(rq == RQ - 1 and not use_lo),
                        )
                    if use_lo:
                        for rq in range(RQ):
                            nc.tensor.matmul(
                                y_ps[:],
                                ust_sb[:, rq, oc * 128 : (oc + 1) * 128],
                                ztlo[:, b, rq, :],
                                start=False,
                                stop=(rq == RQ - 1),
                            )
                    nc.vector.tensor_scalar(
                        y_sb[:, b, :],
                        y_ps[:],
                        s1_sb[:, oc : oc + 1],
                        bias_sb[:, oc : oc + 1],
                        op0=mybir.AluOpType.mult,
                        op1=mybir.AluOpType.add,
                    )
                    if oc == OC - 1:
                        # split the final store so its first half overlaps
                        # the last matmuls (shorter kernel tail)
                        nc.sync.dma_start(
                            out=yt_r[oc, :, b * BLK : (b + 1) * BLK],
                            in_=y_sb[:, b, :],
                        )
                if oc < OC - 1:
                    nc.sync.dma_start(out=yt_r[oc, :, :], in_=y_sb[:, :, :])
    _spill_excess_waits(nc)
    return nc


def _to_pdim(a: np.ndarray, nchunk: int) -> np.ndarray:
    """[nchunk*128, F] row-major -> [128, nchunk*F] with per-partition
    layout [chunk][F] (partition p holds rows {chunk*128 + p})."""
    n, f = a.shape
    assert n == nchunk * 128
    return np.ascontiguousarray(
        a.reshape(nchunk, 128, f).transpose(1, 0, 2)
    ).reshape(128, nchunk * f)


_PROG_CACHE: dict[bool, bass.Bass] = {}


def kernel(x, U_latent, V_latent, s1, s2, bias, _want_trace: bool = False):
    use_lo = USE_LO
    x = np.asarray(x, np.float32)
    s1 = np.asarray(s1, np.float32)
    s2 = np.asarray(s2, np.float32)
    bias = np.asarray(bias, np.float32)

    xs = x.reshape(ROWS, D_IN) * s2[None, :]
    xsT = np.ascontiguousarray(xs.T)  # [D_IN, ROWS] f32

    vs_host = _to_pdim(np.sign(V_latent).astype(BF16), KC)
    ust_host = _to_pdim(
        np.ascontiguousarray(np.sign(U_latent).T).astype(BF16), RQ
    )
    s1_host = np.ascontiguousarray(s1.reshape(OC, 128).T)
    bias_host = np.ascontiguousarray(bias.reshape(OC, 128).T)

    in_maps = []
    for c in range(N_CORES):
        xc = xsT[:, c * RPC : (c + 1) * RPC]  # [D_IN, RPC] f32
        hi = xc.astype(BF16)
        m = {
            "xt": _to_pdim(hi, KC),
            "vs": vs_host,
            "ust": ust_host,
            "s1": s1_host,
            "biast": bias_host,
        }
        if use_lo:
            m["xtlo"] = _to_pdim((xc - hi.astype(np.float32)).astype(BF16), KC)
        in_maps.append(m)

    if use_lo not in _PROG_CACHE:
        _PROG_CACHE[use_lo] = build_program(use_lo)
    nc = _PROG_CACHE[use_lo]

    out = run_bass_kernel_spmd(
        nc, in_maps, core_ids=list(range(N_CORES)), trace=_want_trace
    )
    y = np.empty((ROWS, D_OUT), np.float32)
    for c in range(N_CORES):
        y[c * RPC : (c + 1) * RPC, :] = out.results[c]["yt"].T
    y = y.reshape(B, S, D_OUT)
    if _want_trace:
        return y, out
    return y
